# revision 1
# baseline (speedup 1.0000x reference)
"""Trainium2 Bass kernel for nn_Correlation (stereo cost volume).

  out[b, d, h, w] = mean_c( x[b,c,h,w] * y[b,c,h,w-d] ),  w >= d else 0
  B=8, C=32, H=256, W=512, D=48  (maxdisp=48)

Sharding: data-parallel over batch B across the 8 NeuronCores (one batch
element per core).  Each core computes its full [D, H, W] cost volume.

Per-core algorithm (fp32 compute, fp16 staging/output storage):
  - x/y rows are staged in SBUF in two 32-partition slabs (partitions
    0-31 and 64-95) so consecutive matmuls alternate PE row groups and
    LDWEIGHTS overlaps the running matmul.  y rows are stored
    contiguously with a 47-col lead so a single DMA per slab loads all
    G rows (windows that read across row boundaries only feed the w<d
    zone, which is zeroed later).
  - Per (h, 128-col w-tile): one PE matmul, K=C=32, stationary = X
    columns [32,128], moving = Y window [32,175].  psum[j, u] =
    <x_col(w0+j), y_col(w0+u-47)>, so the 48 outputs of column j sit on
    the diagonal u = j..j+47 (d = j+47-u).
  - DVE/ACT copies scale psum by 1/32 (the exact mean) into an SBUF
    G tile stored as fp16 (halves all downstream traffic; ~5e-4 rel
    error, values are O(1) means of N(0,1) products so no overflow);
    the w<d zone (cols 0:47 of w-tile 0) is zeroed.
  - G is dumped contiguously to a DRAM scratch, and a skewed
    DRAM->DRAM DMA (DRAM is linear, so arbitrary strides are legal -
    SBUF-side skewed access patterns mis-lower in the DGE descriptor
    generation, resetting the per-partition byte skew every 4
    partitions) walks the band diagonals straight into the output in
    [h, w, d_rev] layout with fully contiguous 98KB writes per row.
  - The host casts back to fp32, reverses d and transposes to
    [d, h, w].
"""

import sys

sys.path.insert(0, "/opt/trn_rl_repo")

import numpy as np
from contextlib import ExitStack

import concourse.bass as bass
import concourse.tile as tile
from concourse import mybir
from concourse import bass_utils

B = 8
C = 32
H = 256
W = 512
D = 48
NW = W // 128           # 4 w-tiles per row
MMN = 128 + D - 1       # 175 moving columns per matmul
LEAD = D - 1            # 47
GW = NW * MMN           # 700 G cols per h
G = 16                  # h rows per slab per iteration


def _split_waits(nc, max_waits=1):
    """Walrus codegen accepts at most ONE sync wait per instruction; Tile
    attaches several.  Split extra waits onto preceding NoOps on the same
    engine queue (dispatch is in-order, waits gate dispatch)."""
    for fn in nc.m.functions:
        for blk in fn.blocks:
            newl = []
            changed = False
            for inst in blk.instructions:
                si = getattr(inst, "sync_info", None)
                ow = list(si.on_wait) if si is not None and si.on_wait else []
                if len(ow) > max_waits and inst.engine is not None:
                    for k, wcond in enumerate(ow[:-max_waits]):
                        newl.append(mybir.InstNoOp(
                            name=f"{inst.name}w{k}",
                            engine=inst.engine,
                            sync_info=mybir.SyncInfo(on_wait=[wcond],
                                                     on_update=[]),
                        ))
                    inst.sync_info = mybir.SyncInfo(
                        on_wait=ow[-max_waits:],
                        on_update=list(si.on_update) if si.on_update else [])
                    changed = True
                newl.append(inst)
            if changed:
                blk.instructions = newl


def _emit_body(ctx, tc, x_ap, y_ap, o_ap, act_frac=0.34):
    nc = tc.nc
    n_iter = H // (2 * G)
    o_t = o_ap.tensor
    yflat = y_ap.rearrange("c h w -> c (h w)")

    # DRAM scratch: one [128, 2*GW] block per h-pair
    gd = nc.dram_tensor("gd", [(H // 2) * 128 * 2 * GW], mybir.dt.float16,
                        kind="Internal")

    xpool = ctx.enter_context(tc.tile_pool(name="xp", bufs=2))
    ypool = ctx.enter_context(tc.tile_pool(name="yp", bufs=2))
    gpool = ctx.enter_context(tc.tile_pool(name="gp", bufs=3))
    ppool = ctx.enter_context(tc.tile_pool(name="pp", bufs=6, space="PSUM"))

    inv_c = 1.0 / C
    hcount = 0

    for it in range(n_iter):
        h0 = it * 2 * G
        xt = xpool.tile([128, G * W], mybir.dt.float32, name=f"xt{it}", tag="xt")
        yt = ypool.tile([128, LEAD + G * W], mybir.dt.float32,
                        name=f"yt{it}", tag="yt")

        nc.sync.dma_start(xt[0:C, :], x_ap[:, h0:h0 + G, :])
        nc.sync.dma_start(xt[64:64 + C, :], x_ap[:, h0 + G:h0 + 2 * G, :])
        if it == 0:
            # no rows before row 0: lead cols stay unloaded; the very first
            # w-tile uses a shrunk moving window instead
            nc.sync.dma_start(yt[0:C, LEAD:], yflat[:, 0:G * W])
        else:
            nc.sync.dma_start(yt[0:C, :], yflat[:, h0 * W - LEAD:(h0 + G) * W])
        nc.sync.dma_start(yt[64:64 + C, :],
                          yflat[:, (h0 + G) * W - LEAD:(h0 + 2 * G) * W])

        for g in range(G):
            hs = (h0 + g, h0 + G + g)
            bases = (0, 64)
            gt = gpool.tile([128, 2 * GW], mybir.dt.float16,
                            name=f"gt{it}_{g}", tag="gt")
            psums = []
            for half in range(NW // 2):           # psum pair = 2 w-tiles
                ps = [
                    ppool.tile([128, 2 * MMN], mybir.dt.float32,
                               name=f"ps{it}_{g}_{half}_{s}", tag="ps",
                               padded_shape=[128, 512])
                    for s in range(2)
                ]
                for wsub in range(2):
                    wt = half * 2 + wsub
                    for s in range(2):
                        base = bases[s]
                        lhs = xt[base:base + C,
                                 g * W + wt * 128: g * W + wt * 128 + 128]
                        lo = LEAD if (it == 0 and g == 0 and s == 0
                                      and wt == 0) else 0
                        rhs = yt[base:base + C,
                                 g * W + wt * 128 + lo: g * W + wt * 128 + MMN]
                        nc.tensor.matmul(
                            ps[s][:, wsub * MMN + lo:(wsub + 1) * MMN],
                            lhs, rhs, start=True, stop=True)
                psums.append(ps)

            for s in range(2):
                for half in range(NW // 2):
                    lo = LEAD if (it == 0 and g == 0 and s == 0
                                  and half == 0) else 0
                    dst_sl = gt[:, s * GW + half * 2 * MMN + lo:
                                s * GW + (half + 1) * 2 * MMN]
                    src_sl = psums[half][s][:, lo:]
                    if (hcount % 100) < act_frac * 100:
                        nc.scalar.mul(dst_sl, src_sl, inv_c)
                    else:
                        nc.vector.tensor_scalar_mul(dst_sl, src_sl, inv_c)
                # zero the w<d zone (read from left of the row start)
                nc.vector.memset(gt[:, s * GW:s * GW + LEAD], 0.0)
                hcount += 1

            # dump the h-pair G to DRAM scratch (contiguous 717KB)
            pc = it * G + g
            dmp = bass.AP(gd, pc * 128 * 2 * GW, [[2 * GW, 128], [1, 2 * GW]])
            nc.sync.dma_start(dmp, gt[:, :])
            # skewed extraction per h: band diagonals -> [h, w, d_rev]
            # (all DMAs stay on the SP HWDGE ring: moving any to the ACT
            # ring serializes with the scalar-engine psum drains and
            # measured 27% slower)
            for s in range(2):
                h = hs[s]
                src = bass.AP(gd, pc * 128 * 2 * GW + s * GW,
                              [[2 * GW + 1, 128], [MMN, NW], [1, D]])
                dst = bass.AP(o_t, h * W * D,
                              [[D, 128], [128 * D, NW], [1, D]])
                nc.sync.dma_start(dst, src)


def _build_kernel():
    nc = bass.Bass(trn_type="TRN2", target_bir_lowering=False)
    x_d = nc.dram_tensor("x", [C, H, W], mybir.dt.float32, kind="ExternalInput")
    y_d = nc.dram_tensor("y", [C, H, W], mybir.dt.float32, kind="ExternalInput")
    o_d = nc.dram_tensor("o", [H, W, D], mybir.dt.float16,
                          kind="ExternalOutput")
    with ExitStack() as ctx:
        tc = ctx.enter_context(tile.TileContext(nc))
        _emit_body(ctx, tc, x_d.ap(), y_d.ap(), o_d.ap())
    _split_waits(nc)
    return nc


_NC_CACHE = None


def _get_nc():
    global _NC_CACHE
    if _NC_CACHE is None:
        _NC_CACHE = _build_kernel()
    return _NC_CACHE


def kernel(x: np.ndarray, y: np.ndarray, maxdisp=48) -> np.ndarray:
    assert int(maxdisp) == D
    x = np.ascontiguousarray(np.asarray(x, dtype=np.float32))
    y = np.ascontiguousarray(np.asarray(y, dtype=np.float32))
    assert x.shape == (B, C, H, W) and y.shape == (B, C, H, W)

    nc = _get_nc()
    in_maps = [{"x": x[b], "y": y[b]} for b in range(B)]
    res = bass_utils.run_bass_kernel_spmd(nc, in_maps, core_ids=list(range(B)))

    out = np.empty((B, D, H, W), dtype=np.float32)
    for b in range(B):
        ob = np.asarray(res.results[b]["o"], dtype=np.float32)
        out[b] = ob[:, :, ::-1].transpose(2, 0, 1)   # undo d reversal
    return out


if __name__ == "__main__":
    rng = np.random.default_rng(0)
    x = rng.standard_normal((B, C, H, W), dtype=np.float32)
    y = rng.standard_normal((B, C, H, W), dtype=np.float32)
    out = kernel(x=x, y=y, maxdisp=D)
    print("kernel output:", out.shape, out.dtype)



# revision 2
# speedup vs baseline: 1.6306x; 1.6306x over previous
"""Trainium2 Bass kernel for nn_Correlation (stereo cost volume).

  out[b, d, h, w] = mean_c( x[b,c,h,w] * y[b,c,h,w-d] ),  w >= d else 0
  B=8, C=32, H=256, W=512, D=48  (maxdisp=48)

Sharding: data-parallel over batch B across the 8 NeuronCores (one batch
element per core).  Each core computes its full [D, H, W] cost volume.

Per-core algorithm (bf16 matmul, fp16 band output):
  - The host pre-scales x by 1/C and casts x/y to bf16 (exponent-shift,
    so the fold is exact); input HBM traffic halves and the PE runs at
    1 cycle/row instead of fp32's 4.
  - x/y rows are staged in SBUF in two 32-partition slabs (partitions
    0-31 and 64-95) so consecutive matmuls alternate PE row groups and
    LDWEIGHTS overlaps the running matmul.  y rows are stored
    contiguously with a 47-col lead so a single DMA per slab loads all
    G rows.
  - Per (h, 128-col w-tile): one PE matmul, K=C=32, stationary = X
    columns [32,128], moving = Y window [32,175].  psum[j, u] =
    <x_col(w0+j), y_col(w0+u-47)>, so the 48 outputs of column j sit on
    the diagonal u = j..j+47 (d = j+47-u).
  - DVE/ACT copies move the psum band to an SBUF fp16 tile, and ONE DMA
    per h-pair writes the raw band straight to the output DRAM tensor
    (contiguous 358KB).  No DRAM scratch, no skewed extraction.
  - The HOST extracts the 48 diagonals from the band with a single
    as_strided view (+1 element skew per w position), casts to fp32,
    and zeroes the invalid w < d region.
"""

import sys

sys.path.insert(0, "/opt/trn_rl_repo")

import numpy as np
import ml_dtypes
from contextlib import ExitStack

import concourse.bass as bass
import concourse.tile as tile
from concourse import mybir
from concourse import bass_utils

B = 8
C = 32
H = 256
W = 512
D = 48
NW = W // 128           # 4 w-tiles per row
MMN = 128 + D - 1       # 175 moving columns per matmul
LEAD = D - 1            # 47
GW = NW * MMN           # 700 band cols per h
G = 16                  # h rows per slab per iteration


def _split_waits(nc, max_waits=1):
    """Walrus codegen accepts at most ONE sync wait per instruction; Tile
    attaches several.  Split extra waits onto preceding NoOps on the same
    engine queue (dispatch is in-order, waits gate dispatch)."""
    for fn in nc.m.functions:
        for blk in fn.blocks:
            newl = []
            changed = False
            for inst in blk.instructions:
                si = getattr(inst, "sync_info", None)
                ow = list(si.on_wait) if si is not None and si.on_wait else []
                if len(ow) > max_waits and inst.engine is not None:
                    for k, wcond in enumerate(ow[:-max_waits]):
                        newl.append(mybir.InstNoOp(
                            name=f"{inst.name}w{k}",
                            engine=inst.engine,
                            sync_info=mybir.SyncInfo(on_wait=[wcond],
                                                     on_update=[]),
                        ))
                    inst.sync_info = mybir.SyncInfo(
                        on_wait=ow[-max_waits:],
                        on_update=list(si.on_update) if si.on_update else [])
                    changed = True
                newl.append(inst)
            if changed:
                blk.instructions = newl


def _emit_body(ctx, tc, x_ap, y_ap, o_ap, act_frac=0.5):
    nc = tc.nc
    n_iter = H // (2 * G)
    o_t = o_ap.tensor
    yflat = y_ap.rearrange("c h w -> c (h w)")

    xpool = ctx.enter_context(tc.tile_pool(name="xp", bufs=2))
    ypool = ctx.enter_context(tc.tile_pool(name="yp", bufs=2))
    gpool = ctx.enter_context(tc.tile_pool(name="gp", bufs=3))
    ppool = ctx.enter_context(tc.tile_pool(name="pp", bufs=6, space="PSUM"))

    hcount = 0

    for it in range(n_iter):
        h0 = it * 2 * G
        xt = xpool.tile([128, G * W], mybir.dt.bfloat16, name=f"xt{it}",
                        tag="xt")
        yt = ypool.tile([128, LEAD + G * W], mybir.dt.bfloat16,
                        name=f"yt{it}", tag="yt")

        nc.sync.dma_start(xt[0:C, :], x_ap[:, h0:h0 + G, :])
        nc.sync.dma_start(xt[64:64 + C, :], x_ap[:, h0 + G:h0 + 2 * G, :])
        if it == 0:
            # no rows before row 0: zero the lead cols (their products land
            # only in the invalid w<d zone, but keep them finite)
            nc.vector.memset(yt[0:C, 0:LEAD], 0.0)
            nc.sync.dma_start(yt[0:C, LEAD:], yflat[:, 0:G * W])
        else:
            nc.sync.dma_start(yt[0:C, :], yflat[:, h0 * W - LEAD:(h0 + G) * W])
        nc.sync.dma_start(yt[64:64 + C, :],
                          yflat[:, (h0 + G) * W - LEAD:(h0 + 2 * G) * W])

        for g in range(G):
            bases = (0, 64)
            gt = gpool.tile([128, 2 * GW], mybir.dt.float16,
                            name=f"gt{it}_{g}", tag="gt")
            psums = []
            for half in range(NW // 2):           # psum pair = 2 w-tiles
                ps = [
                    ppool.tile([128, 2 * MMN], mybir.dt.float32,
                               name=f"ps{it}_{g}_{half}_{s}", tag="ps",
                               padded_shape=[128, 512])
                    for s in range(2)
                ]
                for wsub in range(2):
                    wt = half * 2 + wsub
                    for s in range(2):
                        base = bases[s]
                        lhs = xt[base:base + C,
                                 g * W + wt * 128: g * W + wt * 128 + 128]
                        rhs = yt[base:base + C,
                                 g * W + wt * 128: g * W + wt * 128 + MMN]
                        nc.tensor.matmul(
                            ps[s][:, wsub * MMN:(wsub + 1) * MMN],
                            lhs, rhs, start=True, stop=True)
                psums.append(ps)

            # drain psum band -> fp16 SBUF tile (split over ACT and DVE)
            for s in range(2):
                for half in range(NW // 2):
                    dst_sl = gt[:, s * GW + half * 2 * MMN:
                                s * GW + (half + 1) * 2 * MMN]
                    src_sl = psums[half][s][:, :]
                    if (hcount % 100) < act_frac * 100:
                        nc.scalar.copy(dst_sl, src_sl)
                    else:
                        nc.vector.tensor_scalar_mul(dst_sl, src_sl, 1.0)
                    hcount += 1

            # one contiguous band write per h-pair, direct to output
            pc = it * G + g
            dst = bass.AP(o_t, pc * 128 * 2 * GW, [[2 * GW, 128], [1, 2 * GW]])
            nc.sync.dma_start(dst, gt[:, :])


def _build_kernel():
    nc = bass.Bass(trn_type="TRN2", target_bir_lowering=False)
    x_d = nc.dram_tensor("x", [C, H, W], mybir.dt.bfloat16,
                         kind="ExternalInput")
    y_d = nc.dram_tensor("y", [C, H, W], mybir.dt.bfloat16,
                         kind="ExternalInput")
    o_d = nc.dram_tensor("o", [H // 2, 128, 2 * GW], mybir.dt.float16,
                         kind="ExternalOutput")
    with ExitStack() as ctx:
        tc = ctx.enter_context(tile.TileContext(nc))
        _emit_body(ctx, tc, x_d.ap(), y_d.ap(), o_d.ap())
    _split_waits(nc)
    return nc


_NC_CACHE = None


def _get_nc():
    global _NC_CACHE
    if _NC_CACHE is None:
        _NC_CACHE = _build_kernel()
    return _NC_CACHE


def _prep_inputs(x: np.ndarray, y: np.ndarray):
    """Cast to bf16 with the 1/C mean folded into x (exact exponent shift)."""
    xs = (np.asarray(x, dtype=np.float32) * np.float32(1.0 / C)).astype(
        ml_dtypes.bfloat16)
    ys = np.asarray(y, dtype=np.float32).astype(ml_dtypes.bfloat16)
    return np.ascontiguousarray(xs), np.ascontiguousarray(ys)


def _deskew(band: np.ndarray) -> np.ndarray:
    """band: [H//2, 128, 2*GW] fp16 -> [D, H, W] fp32 (w<d left unmasked)."""
    el = band.strides[-1]           # fp16 itemsize
    assert band.flags["C_CONTIGUOUS"]
    # view[pc, s, wt, j, d] = band[pc, j, s*GW + wt*MMN + j + LEAD - d]
    view = np.lib.stride_tricks.as_strided(
        band[:, :, LEAD:],
        shape=(H // 2, 2, NW, 128, D),
        strides=(band.strides[0], GW * el, MMN * el,
                 band.strides[1] + el, -el),
    )
    out = view.astype(np.float32)                       # gather-copy
    out = out.transpose(4, 0, 1, 2, 3).reshape(D, H, W)  # [d, (pc,s), w]
    return out


# h row index for each flattened (pc, s): h = (pc//G)*2G + s*G + (pc%G)
_PC = np.arange(H // 2)
_H_IDX = np.stack([(_PC // G) * 2 * G + s * G + (_PC % G)
                   for s in range(2)], axis=1).reshape(-1)


def kernel(x: np.ndarray, y: np.ndarray, maxdisp=48) -> np.ndarray:
    assert int(maxdisp) == D
    xs, ys = _prep_inputs(x, y)
    assert xs.shape == (B, C, H, W) and ys.shape == (B, C, H, W)

    nc = _get_nc()
    in_maps = [{"x": xs[b], "y": ys[b]} for b in range(B)]
    res = bass_utils.run_bass_kernel_spmd(nc, in_maps, core_ids=list(range(B)))

    out = np.empty((B, D, H, W), dtype=np.float32)
    for b in range(B):
        band = np.asarray(res.results[b]["o"])
        dsk = _deskew(band)                    # [D, H(pc,s order), W]
        out[b][:, _H_IDX, :] = dsk
    # zero the invalid w < d zone
    for d in range(1, D):
        out[:, d, :, :d] = 0.0
    return out


if __name__ == "__main__":
    rng = np.random.default_rng(0)
    x = rng.standard_normal((B, C, H, W), dtype=np.float32)
    y = rng.standard_normal((B, C, H, W), dtype=np.float32)
    out = kernel(x=x, y=y, maxdisp=D)
    print("kernel output:", out.shape, out.dtype)


# revision 5
# speedup vs baseline: 2.0562x; 1.2610x over previous
"""Trainium2 Bass kernel for nn_Correlation (stereo cost volume).

  out[b, d, h, w] = mean_c( x[b,c,h,w] * y[b,c,h,w-d] ),  w >= d else 0
  B=8, C=32, H=256, W=512, D=48  (maxdisp=48)

Sharding: data-parallel over batch B across the 8 NeuronCores (one batch
element per core).  Each core computes its full [D, H, W] cost volume.

Per-core algorithm (bf16 matmul, fp16 band output):
  - The host pre-scales x by 1/C and casts x/y to bf16 (exponent shift,
    so the fold is exact); input HBM traffic halves vs fp32.
  - 4 h-rows are packed per matmul via a BLOCK-DIAGONAL stationary:
    K = 4*C = 128 (partition (hb,c) holds row h=4g+hb), so the PE runs
    at full clock (K=32 matmuls throttle to half clock via the HAM
    activity monitor) and the moving window per 32-wide w-subtile is
    only 79 cols: 1264 streamed cols per 4 rows vs 2800 for the naive
    K=32 scheme.
  - The stationary tile (x, interleaved (st, hb, j) with zero
    off-diagonal blocks) is built by one 512-col DVE copy per group
    from a plainly-DMA'd x tile; the zero blocks are memset ONCE per
    double-buffer and never dirtied (each group's copy overwrites
    exactly the diagonal blocks).
  - 16 matmuls per group write bank-packed PSUM ([128,79] at col
    offsets 0,79,...,395 inside [128,474]/[128,316] tiles); ACT+DVE
    drain them to an fp16 SBUF band tile; ONE DMA per group writes the
    [128,1264] band (2528B/partition) straight to the output.
  - The HOST extracts the 48 diagonals (band col = st*79 + j+47-d at
    partition hb*32+j) with one as_strided view, casts to fp32, and
    zeroes the invalid w < d region.
"""

import sys

sys.path.insert(0, "/opt/trn_rl_repo")

import numpy as np
import ml_dtypes
from contextlib import ExitStack

import concourse.bass as bass
import concourse.tile as tile
from concourse import mybir
from concourse import bass_utils

B = 8
C = 32
H = 256
W = 512
D = 48
LEAD = D - 1            # 47
HB = 4                  # h rows packed per group (K = HB*C = 128)
NG = H // HB            # 64 groups
ST = 16                 # 32-col w-subtiles per row
SW = W // ST            # 32 subtile width
MN = SW + LEAD          # 79 moving cols per subtile matmul
GBW = ST * MN           # 1264 band cols per group
PACK = 6                # psum tiles packed per PSUM bank (6*79*4B < 2KB)


def _split_waits(nc, max_waits=1):
    """Walrus codegen accepts at most ONE sync wait per instruction; Tile
    attaches several.  Split extra waits onto preceding NoOps on the same
    engine queue (dispatch is in-order, waits gate dispatch)."""
    for fn in nc.m.functions:
        for blk in fn.blocks:
            newl = []
            changed = False
            for inst in blk.instructions:
                si = getattr(inst, "sync_info", None)
                ow = list(si.on_wait) if si is not None and si.on_wait else []
                if len(ow) > max_waits and inst.engine is not None:
                    for k, wcond in enumerate(ow[:-max_waits]):
                        newl.append(mybir.InstNoOp(
                            name=f"{inst.name}w{k}",
                            engine=inst.engine,
                            sync_info=mybir.SyncInfo(on_wait=[wcond],
                                                     on_update=[]),
                        ))
                    inst.sync_info = mybir.SyncInfo(
                        on_wait=ow[-max_waits:],
                        on_update=list(si.on_update) if si.on_update else [])
                    changed = True
                newl.append(inst)
            if changed:
                blk.instructions = newl


def _emit_body(ctx, tc, x_ap, y_ap, o_ap):
    nc = tc.nc
    o_t = o_ap.tensor
    x_t = x_ap.tensor
    y_t = y_ap.tensor

    xspool = ctx.enter_context(tc.tile_pool(name="xs", bufs=1))
    xppool = ctx.enter_context(tc.tile_pool(name="xp", bufs=3))
    ypool = ctx.enter_context(tc.tile_pool(name="yp", bufs=3))
    gpool = ctx.enter_context(tc.tile_pool(name="gp", bufs=3))
    ppool = ctx.enter_context(tc.tile_pool(name="pp", bufs=6, space="PSUM"))

    # persistent double-buffered stationary tiles; zero blocks memset once
    xst = [xspool.tile([128, ST * 128], mybir.dt.bfloat16,
                       name=f"xst{i}", tag=f"xst{i}") for i in range(2)]
    nc.vector.memset(xst[0][:, :], 0.0)
    nc.vector.memset(xst[1][:, :], 0.0)

    for g in range(NG):
        sg = xst[g % 2]
        # ---- stage x plain: partition (hb,c) <- x[c, 4g+hb, :] ----
        xp = xppool.tile([128, W], mybir.dt.bfloat16, name=f"xp{g}", tag="xp")
        src = bass.AP(x_t, HB * g * W, [[W, HB], [H * W, C], [1, W]])
        nc.sync.dma_start(xp[:, :], src)
        # ---- interleave into block-diagonal stationary layout ----
        # sg[p=(hb,c), st*128 + hb*32 + j] = xp[p, st*32 + j]; one copy per
        # hb block (the hb*32 col offset is per-partition-group, which a
        # single AP cannot express)
        sg_t = sg[:, :].tensor
        xp_t = xp[:, :].tensor
        for hb in range(HB):
            dst = bass.AP(sg_t, hb * C * (ST * 128) + hb * SW,
                          [[ST * 128, C], [128, ST], [1, SW]])
            srcb = bass.AP(xp_t, hb * C * W,
                           [[W, C], [SW, ST], [1, SW]])
            nc.vector.tensor_scalar_mul(dst, srcb, 1.0)

        # ---- stage y with 47-col lead: partition (hb,c) <- y row ----
        yt = ypool.tile([128, LEAD + W], mybir.dt.bfloat16,
                        name=f"yt{g}", tag="yt")
        if g == 0:
            nc.vector.memset(yt[0:C, 0:LEAD], 0.0)
            nc.sync.dma_start(yt[0:C, LEAD:],
                              bass.AP(y_t, 0, [[H * W, C], [1, W]]))
            nc.sync.dma_start(
                yt[C:128, :],
                bass.AP(y_t, W - LEAD,
                        [[W, HB - 1], [H * W, C], [1, LEAD + W]]))
        else:
            nc.sync.dma_start(
                yt[:, :],
                bass.AP(y_t, HB * g * W - LEAD,
                        [[W, HB], [H * W, C], [1, LEAD + W]]))

        # ---- 16 subtile matmuls into 3 bank-packed psum tiles ----
        nps = [PACK, PACK, ST - 2 * PACK]
        pst = [ppool.tile([128, n * MN], mybir.dt.float32,
                          name=f"ps{g}_{i}", tag="ps",
                          padded_shape=[128, 512])
               for i, n in enumerate(nps)]
        for st in range(ST):
            ti, off = divmod(st, PACK)
            nc.tensor.matmul(
                pst[ti][:, off * MN:(off + 1) * MN],
                sg[:, st * 128:(st + 1) * 128],
                yt[:, st * SW: st * SW + MN],
                start=True, stop=True)

        # ---- drain band to fp16 (ACT; DVE is busy with interleaves) ----
        gt = gpool.tile([128, GBW], mybir.dt.float16, name=f"gt{g}", tag="gt")
        c0 = PACK * MN
        c1 = 2 * PACK * MN
        nc.scalar.copy(gt[:, 0:c0], pst[0][:, :])
        nc.scalar.copy(gt[:, c0:c1], pst[1][:, :])
        nc.scalar.copy(gt[:, c1:GBW], pst[2][:, :])

        # ---- one contiguous band write per group ----
        dst = bass.AP(o_t, g * 128 * GBW, [[GBW, 128], [1, GBW]])
        nc.sync.dma_start(dst, gt[:, :])


def _build_kernel():
    nc = bass.Bass(trn_type="TRN2", target_bir_lowering=False)
    x_d = nc.dram_tensor("x", [C, H, W], mybir.dt.bfloat16,
                         kind="ExternalInput")
    y_d = nc.dram_tensor("y", [C, H, W], mybir.dt.bfloat16,
                         kind="ExternalInput")
    o_d = nc.dram_tensor("o", [NG, 128, GBW], mybir.dt.float16,
                         kind="ExternalOutput")
    with ExitStack() as ctx:
        tc = ctx.enter_context(tile.TileContext(nc))
        _emit_body(ctx, tc, x_d.ap(), y_d.ap(), o_d.ap())
    _split_waits(nc)
    return nc


_NC_CACHE = None


def _get_nc():
    global _NC_CACHE
    if _NC_CACHE is None:
        _NC_CACHE = _build_kernel()
    return _NC_CACHE


def _prep_inputs(x: np.ndarray, y: np.ndarray):
    """Cast to bf16 with the 1/C mean folded into x (exact exponent shift)."""
    xs = (np.asarray(x, dtype=np.float32) * np.float32(1.0 / C)).astype(
        ml_dtypes.bfloat16)
    ys = np.asarray(y, dtype=np.float32).astype(ml_dtypes.bfloat16)
    return np.ascontiguousarray(xs), np.ascontiguousarray(ys)


def _deskew(band: np.ndarray) -> np.ndarray:
    """band: [NG, 128, GBW] fp16 -> [D, H, W] fp32 (w<d left unmasked)."""
    el = band.strides[-1]
    assert band.flags["C_CONTIGUOUS"]
    # view[g, hb, j, st, d] = band[g, hb*32+j, st*MN + j + LEAD - d]
    view = np.lib.stride_tricks.as_strided(
        band[:, :, LEAD:],
        shape=(NG, HB, SW, ST, D),
        strides=(band.strides[0], SW * band.strides[1], band.strides[1] + el,
                 MN * el, -el),
    )
    out = view.astype(np.float32)
    # [d, (g, hb), (st, j)] = [d, h, w]
    out = out.transpose(4, 0, 1, 3, 2).reshape(D, H, W)
    return out


def kernel(x: np.ndarray, y: np.ndarray, maxdisp=48) -> np.ndarray:
    assert int(maxdisp) == D
    xs, ys = _prep_inputs(x, y)
    assert xs.shape == (B, C, H, W) and ys.shape == (B, C, H, W)

    nc = _get_nc()
    in_maps = [{"x": xs[b], "y": ys[b]} for b in range(B)]
    res = bass_utils.run_bass_kernel_spmd(nc, in_maps, core_ids=list(range(B)))

    out = np.empty((B, D, H, W), dtype=np.float32)
    for b in range(B):
        band = np.asarray(res.results[b]["o"])
        out[b] = _deskew(band)
    # zero the invalid w < d zone
    for d in range(1, D):
        out[:, d, :, :d] = 0.0
    return out


if __name__ == "__main__":
    rng = np.random.default_rng(0)
    x = rng.standard_normal((B, C, H, W), dtype=np.float32)
    y = rng.standard_normal((B, C, H, W), dtype=np.float32)
    out = kernel(x=x, y=y, maxdisp=D)
    print("kernel output:", out.shape, out.dtype)


# revision 8
# speedup vs baseline: 2.1160x; 1.0291x over previous
"""Trainium2 Bass kernel for nn_Correlation (stereo cost volume).

  out[b, d, h, w] = mean_c( x[b,c,h,w] * y[b,c,h,w-d] ),  w >= d else 0
  B=8, C=32, H=256, W=512, D=48  (maxdisp=48)

Sharding: data-parallel over batch B across the 8 NeuronCores (one batch
element per core).  Each core computes its full [D, H, W] cost volume.

Per-core algorithm (bf16 matmul, fp16 band output):
  - The host pre-scales x by 1/C and casts x/y to bf16 (exponent shift,
    so the fold is exact); input HBM traffic halves vs fp32.
  - 4 h-rows are packed per matmul via a BLOCK-DIAGONAL stationary:
    K = 4*C = 128 (the PE HAM clock-gates K=32 matmuls to half clock;
    K=128 runs at 2.4 GHz) and the moving window per 32-wide w-subtile
    is 79 cols: 1264 streamed cols per 4 rows vs 2800 for K=32.
  - A group packs rows {g, 64+g, 128+g, 192+g} (stride 64, NOT
    consecutive): partition (hb,c) then sees CONTIGUOUS DRAM across
    consecutive groups, so ONE 3-dim DMA stages x (and y, lead
    included) for 8 groups at 8KB/partition -- 8 input triggers total
    per tensor instead of 64.
  - The stationary tiles (x, interleaved (st, hb, j) with zero
    off-diagonal blocks) are built by four 512-col DVE copies per group
    from the plain x slab; the zero blocks are memset ONCE per buffer
    (4-deep ring) and never dirtied.
  - 16 matmuls per group write bank-packed PSUM ([128,79] at col
    offsets 0,79,...,395 inside [128,474]/[128,316] tiles); ACT (x2)
    and GPSIMD (x1) drain them to fp16 band tiles; one [128,2528] DMA
    per PAIR of groups stores the band (sync HWDGE ring).
  - The HOST extracts the 48 diagonals (band col = st*79 + j+47-d at
    partition hb*32+j, h = 64*hb + g) with one as_strided view, casts
    to fp32, and zeroes the invalid w < d region.
"""

import sys

sys.path.insert(0, "/opt/trn_rl_repo")

import numpy as np
import ml_dtypes
from contextlib import ExitStack

import concourse.bass as bass
import concourse.tile as tile
from concourse import mybir
from concourse import bass_utils

B = 8
C = 32
H = 256
W = 512
D = 48
LEAD = D - 1            # 47
HB = 4                  # h rows packed per group (K = HB*C = 128)
NG = H // HB            # 64 groups; group g = rows {g, 64+g, 128+g, 192+g}
HS = H // HB            # 64: h-stride between the packed rows
ST = 16                 # 32-col w-subtiles per row
SW = W // ST            # 32 subtile width
MN = SW + LEAD          # 79 moving cols per subtile matmul
GBW = ST * MN           # 1264 band cols per group
PACK = 6                # psum tiles packed per PSUM bank (6*79*4B < 2KB)
QS = 8                  # groups staged per input slab DMA
NSLAB = NG // QS        # 8 slabs


def _split_waits(nc, max_waits=1):
    """Walrus codegen accepts at most ONE sync wait per instruction; Tile
    attaches several.  Split extra waits onto preceding NoOps on the same
    engine queue (dispatch is in-order, waits gate dispatch)."""
    for fn in nc.m.functions:
        for blk in fn.blocks:
            newl = []
            changed = False
            for inst in blk.instructions:
                si = getattr(inst, "sync_info", None)
                ow = list(si.on_wait) if si is not None and si.on_wait else []
                if len(ow) > max_waits and inst.engine is not None:
                    for k, wcond in enumerate(ow[:-max_waits]):
                        newl.append(mybir.InstNoOp(
                            name=f"{inst.name}w{k}",
                            engine=inst.engine,
                            sync_info=mybir.SyncInfo(on_wait=[wcond],
                                                     on_update=[]),
                        ))
                    inst.sync_info = mybir.SyncInfo(
                        on_wait=ow[-max_waits:],
                        on_update=list(si.on_update) if si.on_update else [])
                    changed = True
                newl.append(inst)
            if changed:
                blk.instructions = newl


def _emit_body(ctx, tc, x_ap, y_ap, o_ap):
    nc = tc.nc
    o_t = o_ap.tensor
    x_t = x_ap.tensor
    y_t = y_ap.tensor

    xspool = ctx.enter_context(tc.tile_pool(name="xs", bufs=1))
    xppool = ctx.enter_context(tc.tile_pool(name="xp", bufs=2))
    ypool = ctx.enter_context(tc.tile_pool(name="yp", bufs=2))
    gpool = ctx.enter_context(tc.tile_pool(name="gp", bufs=3))
    ppool = ctx.enter_context(tc.tile_pool(name="pp", bufs=6, space="PSUM"))

    XW = QS * W             # 4096 staged x cols per slab
    YW = LEAD + QS * W      # 4143 staged y cols per slab

    # persistent stationary ring; zero blocks memset once, never dirtied
    NXS = 4
    xst = [xspool.tile([128, ST * 128], mybir.dt.bfloat16,
                       name=f"xst{i}", tag=f"xst{i}") for i in range(NXS)]
    for i in range(NXS):
        nc.vector.memset(xst[i][:, :], 0.0)

    for q in range(NSLAB):
        # ---- stage x for 8 groups: partition (hb,c), contiguous rows ----
        xp = xppool.tile([128, XW], mybir.dt.bfloat16, name=f"xp{q}",
                         tag="xp")
        nc.sync.dma_start(
            xp[:, :],
            bass.AP(x_t, QS * q * W,
                    [[HS * W, HB], [H * W, C], [1, XW]]))

        # ---- stage y for 8 groups, 47-col lead included ----
        yt = ypool.tile([128, YW], mybir.dt.bfloat16, name=f"yt{q}",
                        tag="yt")
        if q == 0:
            nc.vector.memset(yt[0:C, 0:LEAD], 0.0)
            nc.sync.dma_start(
                yt[0:C, LEAD:],
                bass.AP(y_t, 0, [[H * W, C], [1, QS * W]]))
            nc.sync.dma_start(
                yt[C:128, :],
                bass.AP(y_t, HS * W - LEAD,
                        [[HS * W, HB - 1], [H * W, C], [1, YW]]))
        else:
            nc.sync.dma_start(
                yt[:, :],
                bass.AP(y_t, QS * q * W - LEAD,
                        [[HS * W, HB], [H * W, C], [1, YW]]))

        xp_t = xp[:, :].tensor

        for kk in range(QS // 2):     # pairs of groups within the slab
            gt = gpool.tile([128, 2 * GBW], mybir.dt.float16,
                            name=f"gt{q}_{kk}", tag="gt")
            for e in range(2):
                gq = 2 * kk + e       # group index within slab
                g = QS * q + gq       # global group index
                sg = xst[g % NXS]
                sg_t = sg[:, :].tensor

                # ---- interleave into block-diagonal stationary ----
                # sg[p=(hb,c), st*128+hb*32+j] = xp[p, gq*512 + st*32+j]
                for hb in range(HB):
                    dst = bass.AP(sg_t, hb * C * (ST * 128) + hb * SW,
                                  [[ST * 128, C], [128, ST], [1, SW]])
                    srcb = bass.AP(xp_t, hb * C * XW + gq * W,
                                   [[XW, C], [SW, ST], [1, SW]])
                    nc.vector.tensor_scalar_mul(dst, srcb, 1.0)

                # ---- 16 subtile matmuls into 3 bank-packed psums ----
                nps = [PACK, PACK, ST - 2 * PACK]
                pst = [ppool.tile([128, n * MN], mybir.dt.float32,
                                  name=f"ps{g}_{i}", tag="ps",
                                  padded_shape=[128, 512])
                       for i, n in enumerate(nps)]
                for st in range(ST):
                    ti, off = divmod(st, PACK)
                    nc.tensor.matmul(
                        pst[ti][:, off * MN:(off + 1) * MN],
                        sg[:, st * 128:(st + 1) * 128],
                        yt[:, gq * W + st * SW: gq * W + st * SW + MN],
                        start=True, stop=True)

                # ---- drain band to fp16 (ACT x2.5, DVE x0.5) ----
                c0 = PACK * MN
                c1 = 2 * PACK * MN
                ch = c1 + 2 * MN
                base = e * GBW
                nc.scalar.copy(gt[:, base:base + c0], pst[0][:, :])
                nc.scalar.copy(gt[:, base + c0:base + c1], pst[1][:, :])
                nc.scalar.copy(gt[:, base + c1:base + ch],
                               pst[2][:, 0:2 * MN])
                nc.vector.tensor_scalar_mul(gt[:, base + ch:base + GBW],
                                            pst[2][:, 2 * MN:], 1.0)

            # ---- one band store per pair (sync HWDGE ring) ----
            g0 = QS * q + 2 * kk
            dst = bass.AP(o_t, g0 * 128 * GBW,
                          [[GBW, 128], [128 * GBW, 2], [1, GBW]])
            nc.sync.dma_start(dst, gt[:, :])


def _build_kernel():
    nc = bass.Bass(trn_type="TRN2", target_bir_lowering=False)
    x_d = nc.dram_tensor("x", [C, H, W], mybir.dt.bfloat16,
                         kind="ExternalInput")
    y_d = nc.dram_tensor("y", [C, H, W], mybir.dt.bfloat16,
                         kind="ExternalInput")
    o_d = nc.dram_tensor("o", [NG, 128, GBW], mybir.dt.float16,
                         kind="ExternalOutput")
    with ExitStack() as ctx:
        tc = ctx.enter_context(tile.TileContext(nc))
        _emit_body(ctx, tc, x_d.ap(), y_d.ap(), o_d.ap())
    _split_waits(nc)
    return nc


_NC_CACHE = None


def _get_nc():
    global _NC_CACHE
    if _NC_CACHE is None:
        _NC_CACHE = _build_kernel()
    return _NC_CACHE


def _prep_inputs(x: np.ndarray, y: np.ndarray):
    """Cast to bf16 with the 1/C mean folded into x (exact exponent shift)."""
    xs = (np.asarray(x, dtype=np.float32) * np.float32(1.0 / C)).astype(
        ml_dtypes.bfloat16)
    ys = np.asarray(y, dtype=np.float32).astype(ml_dtypes.bfloat16)
    return np.ascontiguousarray(xs), np.ascontiguousarray(ys)


def _deskew(band: np.ndarray) -> np.ndarray:
    """band: [NG, 128, GBW] fp16 -> [D, H, W] fp32 (w<d left unmasked)."""
    el = band.strides[-1]
    assert band.flags["C_CONTIGUOUS"]
    # view[g, hb, j, st, d] = band[g, hb*32+j, st*MN + j + LEAD - d]
    view = np.lib.stride_tricks.as_strided(
        band[:, :, LEAD:],
        shape=(NG, HB, SW, ST, D),
        strides=(band.strides[0], SW * band.strides[1], band.strides[1] + el,
                 MN * el, -el),
    )
    out = view.astype(np.float32)
    # h = 64*hb + g:  [d, (hb, g), (st, j)] = [d, h, w]
    out = out.transpose(4, 1, 0, 3, 2).reshape(D, H, W)
    return out


def kernel(x: np.ndarray, y: np.ndarray, maxdisp=48) -> np.ndarray:
    assert int(maxdisp) == D
    xs, ys = _prep_inputs(x, y)
    assert xs.shape == (B, C, H, W) and ys.shape == (B, C, H, W)

    nc = _get_nc()
    in_maps = [{"x": xs[b], "y": ys[b]} for b in range(B)]
    res = bass_utils.run_bass_kernel_spmd(nc, in_maps, core_ids=list(range(B)))

    out = np.empty((B, D, H, W), dtype=np.float32)
    for b in range(B):
        band = np.asarray(res.results[b]["o"])
        out[b] = _deskew(band)
    # zero the invalid w < d zone
    for d in range(1, D):
        out[:, d, :, :d] = 0.0
    return out


if __name__ == "__main__":
    rng = np.random.default_rng(0)
    x = rng.standard_normal((B, C, H, W), dtype=np.float32)
    y = rng.standard_normal((B, C, H, W), dtype=np.float32)
    out = kernel(x=x, y=y, maxdisp=D)
    print("kernel output:", out.shape, out.dtype)


# revision 10
# speedup vs baseline: 3.1166x; 1.4728x over previous
"""Trainium2 Bass kernel for nn_Correlation (stereo cost volume).

  out[b, d, h, w] = mean_c( x[b,c,h,w] * y[b,c,h,w-d] ),  w >= d else 0
  B=8, C=32, H=256, W=512, D=48  (maxdisp=48)

Sharding: data-parallel over batch B across the 8 NeuronCores (one batch
element per core).  Each core computes its full [D, H, W] cost volume.

Per-core algorithm (bf16 matmul, fp16 band output):
  - The host pre-scales x by 1/C, casts to bf16, and PRE-PERMUTES both
    inputs into the SBUF staging layout [128, ...] with partition
    (hb,c) = row 64*hb+g of channel c (y rows carry their 47-col lead
    inline, zero-padded at h=0).  Every DMA is then a plain 2-dim
    (partition x contiguous-run) pattern — the fast "direct 2D" HWDGE
    path — and input HBM traffic is halved vs fp32.
  - 4 h-rows {g, 64+g, 128+g, 192+g} are packed per matmul via a
    BLOCK-DIAGONAL stationary: K = 4*C = 128 (the PE HAM clock-gates
    K=32 matmuls to half clock) and the moving window per 32-wide
    w-subtile is 79 cols: 1264 streamed cols per 4 rows vs 2800 for
    K=32.
  - Stationary tiles (x interleaved (st, hb, j), zero off-diagonal
    blocks) are built by four 512-col DVE copies per group; the zero
    blocks are memset ONCE per buffer (4-ring) and never dirtied.
  - 16 matmuls per group write bank-packed PSUM ([128,79] at col
    offsets 0..395 inside [128,474]/[128,316] tiles); ACT (2.5) and
    DVE (0.5) drain them to fp16 band tiles; one [128,2528] store per
    PAIR of groups.
  - One DMA instruction only engages a 4-engine group (~100 GB/s), so
    slab loads are split into 4x 32-partition chunks and PREFETCHED a
    full slab ahead of the stores on the in-order sync HWDGE ring
    (16 engines, ~400 GB/s, no head-of-line blocking on drain sems).
  - The HOST extracts the 48 diagonals (band col = st*79 + j+47-d at
    partition hb*32+j, h = 64*hb + g) with one as_strided view, casts
    to fp32, and zeroes the invalid w < d region.
"""

import sys

sys.path.insert(0, "/opt/trn_rl_repo")

import numpy as np
import ml_dtypes
from contextlib import ExitStack

import concourse.bass as bass
import concourse.tile as tile
from concourse import mybir
from concourse import bass_utils

B = 8
C = 32
H = 256
W = 512
D = 48
LEAD = D - 1            # 47
HB = 4                  # h rows packed per group (K = HB*C = 128)
NG = H // HB            # 64 groups; group g = rows {g, 64+g, 128+g, 192+g}
HS = H // HB            # 64: h-stride between the packed rows
ST = 16                 # 32-col w-subtiles per row
SW = W // ST            # 32 subtile width
MN = SW + LEAD          # 79 moving cols per subtile matmul
GBW = ST * MN           # 1264 band cols per group
PACK = 6                # psum tiles packed per PSUM bank (6*79*4B < 2KB)
QS = 8                  # groups staged per input slab
NSLAB = NG // QS        # 8 slabs
XROW = HS * W           # 32768 x cols per staged partition row
YROW = LEAD + HS * W    # 32815 y cols per staged partition row
XW = QS * W             # 4096 staged x cols per slab
YW = LEAD + QS * W      # 4143 staged y cols per slab


def _split_waits(nc, max_waits=1):
    """Walrus codegen accepts at most ONE sync wait per instruction; Tile
    attaches several.  Split extra waits onto preceding NoOps on the same
    engine queue (dispatch is in-order, waits gate dispatch)."""
    for fn in nc.m.functions:
        for blk in fn.blocks:
            newl = []
            changed = False
            for inst in blk.instructions:
                si = getattr(inst, "sync_info", None)
                ow = list(si.on_wait) if si is not None and si.on_wait else []
                if len(ow) > max_waits and inst.engine is not None:
                    for k, wcond in enumerate(ow[:-max_waits]):
                        newl.append(mybir.InstNoOp(
                            name=f"{inst.name}w{k}",
                            engine=inst.engine,
                            sync_info=mybir.SyncInfo(on_wait=[wcond],
                                                     on_update=[]),
                        ))
                    inst.sync_info = mybir.SyncInfo(
                        on_wait=ow[-max_waits:],
                        on_update=list(si.on_update) if si.on_update else [])
                    changed = True
                newl.append(inst)
            if changed:
                blk.instructions = newl


def _emit_body(ctx, tc, x_ap, y_ap, o_ap):
    nc = tc.nc
    o_t = o_ap.tensor
    x_t = x_ap.tensor
    y_t = y_ap.tensor

    xspool = ctx.enter_context(tc.tile_pool(name="xs", bufs=1))
    xppool = ctx.enter_context(tc.tile_pool(name="xp", bufs=2))
    ypool = ctx.enter_context(tc.tile_pool(name="yp", bufs=2))
    gpool = ctx.enter_context(tc.tile_pool(name="gp", bufs=3))
    ppool = ctx.enter_context(tc.tile_pool(name="pp", bufs=6, space="PSUM"))

    def load_slab(q):
        """4-chunk 2-dim loads for x and y of slab q (fast DGE path)."""
        xp = xppool.tile([128, XW], mybir.dt.bfloat16, name=f"xp{q}",
                         tag="xp")
        yt = ypool.tile([128, YW], mybir.dt.bfloat16, name=f"yt{q}",
                        tag="yt")
        for m in range(4):
            nc.sync.dma_start(
                xp[32 * m:32 * (m + 1), :],
                bass.AP(x_t, 32 * m * XROW + q * XW, [[XROW, 32], [1, XW]]))
            nc.sync.dma_start(
                yt[32 * m:32 * (m + 1), :],
                bass.AP(y_t, 32 * m * YROW + q * XW, [[YROW, 32], [1, YW]]))
        return xp, yt

    # persistent stationary ring; zero blocks memset once, never dirtied
    NXS = 4
    xst = [xspool.tile([128, ST * 128], mybir.dt.bfloat16,
                       name=f"xst{i}", tag=f"xst{i}") for i in range(NXS)]

    xp, yt = load_slab(0)
    for i in range(NXS):
        if i % 2 == 0:
            nc.vector.memset(xst[i][:, :], 0.0)
        else:
            nc.scalar.copy(xst[i][:, :], xst[i - 1][:, :])

    for q in range(NSLAB):
        if q + 1 < NSLAB:
            nxt = load_slab(q + 1)      # prefetch a full slab ahead
        xp_t = xp[:, :].tensor

        for kk in range(QS // 2):       # pairs of groups within the slab
            gt = gpool.tile([128, 2 * GBW], mybir.dt.float16,
                            name=f"gt{q}_{kk}", tag="gt")
            for e in range(2):
                gq = 2 * kk + e         # group index within slab
                g = QS * q + gq         # global group index
                sg = xst[g % NXS]
                sg_t = sg[:, :].tensor

                # ---- interleave into block-diagonal stationary ----
                # sg[p=(hb,c), st*128+hb*32+j] = xp[p, gq*512 + st*32+j]
                for hb in range(HB):
                    dst = bass.AP(sg_t, hb * C * (ST * 128) + hb * SW,
                                  [[ST * 128, C], [128, ST], [1, SW]])
                    srcb = bass.AP(xp_t, hb * C * XW + gq * W,
                                   [[XW, C], [SW, ST], [1, SW]])
                    nc.vector.tensor_scalar_mul(dst, srcb, 1.0)

                # ---- 16 subtile matmuls into 3 bank-packed psums ----
                nps = [PACK, PACK, ST - 2 * PACK]
                pst = [ppool.tile([128, n * MN], mybir.dt.float32,
                                  name=f"ps{g}_{i}", tag="ps",
                                  padded_shape=[128, 512])
                       for i, n in enumerate(nps)]
                for st in range(ST):
                    ti, off = divmod(st, PACK)
                    nc.tensor.matmul(
                        pst[ti][:, off * MN:(off + 1) * MN],
                        sg[:, st * 128:(st + 1) * 128],
                        yt[:, gq * W + st * SW: gq * W + st * SW + MN],
                        start=True, stop=True)

                # ---- drain band to fp16 (ACT x2.5, DVE x0.5) ----
                c0 = PACK * MN
                c1 = 2 * PACK * MN
                ch = c1 + 2 * MN
                base = e * GBW
                nc.scalar.copy(gt[:, base:base + c0], pst[0][:, :])
                nc.scalar.copy(gt[:, base + c0:base + c1], pst[1][:, :])
                nc.scalar.copy(gt[:, base + c1:base + ch],
                               pst[2][:, 0:2 * MN])
                nc.vector.tensor_scalar_mul(gt[:, base + ch:base + GBW],
                                            pst[2][:, 2 * MN:], 1.0)

            # ---- one band store per pair (after the prefetched loads) ----
            g0 = QS * q + 2 * kk
            dst = bass.AP(o_t, g0 * 128 * GBW,
                          [[GBW, 128], [128 * GBW, 2], [1, GBW]])
            nc.sync.dma_start(dst, gt[:, :])

        if q + 1 < NSLAB:
            xp, yt = nxt


def _build_kernel():
    nc = bass.Bass(trn_type="TRN2", target_bir_lowering=False)
    x_d = nc.dram_tensor("x", [128, XROW], mybir.dt.bfloat16,
                         kind="ExternalInput")
    y_d = nc.dram_tensor("y", [128, YROW], mybir.dt.bfloat16,
                         kind="ExternalInput")
    o_d = nc.dram_tensor("o", [NG, 128, GBW], mybir.dt.float16,
                         kind="ExternalOutput")
    with ExitStack() as ctx:
        tc = ctx.enter_context(tile.TileContext(nc))
        _emit_body(ctx, tc, x_d.ap(), y_d.ap(), o_d.ap())
    _split_waits(nc)
    return nc


_NC_CACHE = None


def _get_nc():
    global _NC_CACHE
    if _NC_CACHE is None:
        _NC_CACHE = _build_kernel()
    return _NC_CACHE


def _prep_inputs(x: np.ndarray, y: np.ndarray):
    """Cast to bf16 (1/C folded into x — exact exponent shift) and
    pre-permute into the staged SBUF layouts:
      x2[b, p=(hb,c), g*512+w]       = (x/C)[b, c, 64*hb+g, w]
      y2[b, p=(hb,c), 47 + gg*512+w] = y[b, c, 64*hb+gg, w]  (lead inline)
    """
    xs = (np.asarray(x, dtype=np.float32) * np.float32(1.0 / C)).astype(
        ml_dtypes.bfloat16)
    ys = np.asarray(y, dtype=np.float32).astype(ml_dtypes.bfloat16)
    # x: [B, C, (hb, 64), W] -> [B, hb, C, 64*W]
    x2 = np.ascontiguousarray(
        xs.reshape(B, C, HB, HS, W).transpose(0, 2, 1, 3, 4)
        .reshape(B, 128, XROW))
    yf = ys.reshape(B, C, H * W)
    y2 = np.empty((B, HB, C, YROW), dtype=ml_dtypes.bfloat16)
    for hb in range(HB):
        s = hb * HS * W
        y2[:, hb, :, LEAD:] = yf[:, :, s:s + HS * W]
        if hb == 0:
            y2[:, 0, :, :LEAD] = ml_dtypes.bfloat16(0.0)
        else:
            y2[:, hb, :, :LEAD] = yf[:, :, s - LEAD:s]
    y2 = np.ascontiguousarray(y2.reshape(B, 128, YROW))
    return x2, y2


def _deskew(band: np.ndarray) -> np.ndarray:
    """band: [NG, 128, GBW] fp16 -> [D, H, W] fp32 (w<d left unmasked)."""
    el = band.strides[-1]
    assert band.flags["C_CONTIGUOUS"]
    # view[g, hb, j, st, d] = band[g, hb*32+j, st*MN + j + LEAD - d]
    view = np.lib.stride_tricks.as_strided(
        band[:, :, LEAD:],
        shape=(NG, HB, SW, ST, D),
        strides=(band.strides[0], SW * band.strides[1], band.strides[1] + el,
                 MN * el, -el),
    )
    out = view.astype(np.float32)
    # h = 64*hb + g:  [d, (hb, g), (st, j)] = [d, h, w]
    out = out.transpose(4, 1, 0, 3, 2).reshape(D, H, W)
    return out


def kernel(x: np.ndarray, y: np.ndarray, maxdisp=48) -> np.ndarray:
    assert int(maxdisp) == D
    x2, y2 = _prep_inputs(x, y)

    nc = _get_nc()
    in_maps = [{"x": x2[b], "y": y2[b]} for b in range(B)]
    res = bass_utils.run_bass_kernel_spmd(nc, in_maps, core_ids=list(range(B)))

    out = np.empty((B, D, H, W), dtype=np.float32)
    for b in range(B):
        band = np.asarray(res.results[b]["o"])
        out[b] = _deskew(band)
    # zero the invalid w < d zone
    for d in range(1, D):
        out[:, d, :, :d] = 0.0
    return out


if __name__ == "__main__":
    rng = np.random.default_rng(0)
    x = rng.standard_normal((B, C, H, W), dtype=np.float32)
    y = rng.standard_normal((B, C, H, W), dtype=np.float32)
    out = kernel(x=x, y=y, maxdisp=D)
    print("kernel output:", out.shape, out.dtype)
